# revision 1
# baseline (speedup 1.0000x reference)
"""MeshPoolTrans: out[b,p,f] = sum_{k: rows[k]==p} vals[k] * x[b,cols[k],f].

Sharding: data-parallel over batch B=16 across 8 cores (2 batches/core);
the sparse pooling structure (rows/cols/vals) is replicated on every core.

Per-core device algorithm:
  1. Host interleaves the core's batches: x2[m] = [x[b0,m,:] | x[b1,m,:]]
     ([M, bpc*F]), so one gather descriptor fetches a mesh vertex's
     features for every local batch at once.
  2. Host sorts nnz by row, buckets them by output tile (128 rows), pads
     each bucket to a multiple of 128 -> "sub-chunks" of 128 nnz.
     dma_gather indices are int16 (< 32768) while cols reach M-1=40961, so
     each bucket's nnz are split between a "lo" window (x2 rows [0, 32768))
     and a "hi" window (x2 rows [M-32768, M)), with the overlap region used
     to balance the two halves and minimize padding.
  3. dma_gather ops of up to OPC sub-chunks (<= 1024 indices, the SWDGE
     descriptor-ring limit) pull x2 rows into SBUF blocks
     G [128 nnz, bpc*F], chunked along each window's global nnz stream.
  4. For each sub-chunk, a one-hot selection matrix S[i,j] =
     (j == row_rel[i]) * val[i] is built with one tensor_scalar op, and
     matmul psum[128 rows, bpc*F] += S.T @ G accumulates the segment sum
     for the output tile; PSUM is copied to SBUF and DMA'd out per batch.
"""

import sys

sys.path.insert(0, "/opt/trn_rl_repo")

import numpy as np

import concourse.bass as bass
import concourse.mybir as mybir
import concourse.tile as tile
from concourse import bacc
from concourse.bass_utils import run_bass_kernel_spmd

P = 128
NCORES = 8
W = 32768  # int16 index window
OPC = 8  # sub-chunks per dma_gather op (8*128 = 1024-desc SWDGE ring limit)
GBUFS = 6  # gather tile ring slots
PSBUFS = 8  # psum tiles in flight
ABLATE = ()  # timing expts: subset of {"gather", "staticg", "compute", "out"}
QUEUES = 1  # SWDGE queues to round-robin gather ops over (ucode max 4)
DTYPE = "float32"  # matmul operand dtype: "float32" or "bfloat16"
SELBUFS = 6  # selection-matrix tile ring slots
STBUFS = 4  # output staging tile ring slots
EAGER = 0  # issue all gather ops ahead of the compute loop


def _cdiv(a, b):
    return (a + b - 1) // b


class Plan:
    pass


def _split_bucket(n_lo_only, n_mid, n_hi_only):
    """Pick how many of the bucket's entries go to the lo window.

    Entries are ordered [lo_only, mid, hi_only]; any prefix of mid can go
    lo. Minimizes ceil(lo/128) + ceil(hi/128)."""
    a, m, h = n_lo_only, n_mid, n_hi_only
    best = None
    cands = {0, m}
    t0 = (-a) % P
    while t0 <= m:
        cands.add(t0)
        t0 += P
    for t in cands:
        lo, hi = a + t, h + (m - t)
        cost = _cdiv(lo, P) + _cdiv(hi, P)
        if best is None or cost < best[0]:
            best = (cost, t)
    return best[1]


def _build_plan(rows, cols, vals, M, MP):
    """Bucket/pad/split the sparse structure into 128-nnz sub-chunks."""
    NT = _cdiv(MP, P)
    hi_base = max(0, M - W)

    order = np.argsort(rows, kind="stable")
    r = np.asarray(rows)[order].astype(np.int64)
    c = np.asarray(cols)[order].astype(np.int64)
    v = np.asarray(vals)[order].astype(np.float32)
    bucket = r // P
    counts = np.bincount(bucket, minlength=NT)
    starts = np.concatenate([[0], np.cumsum(counts)])

    sc_half = []  # per sub-chunk: 0 = lo, 1 = hi
    sc_idx = []  # [128] window-relative gather index
    sc_rel = []  # [128] row - 128*q
    sc_val = []  # [128] value
    bucket_sc = []  # (q, sub-chunk id list)
    for q in range(NT):
        lo, hi = starts[q], starts[q + 1]
        cq, rq, vq = c[lo:hi], r[lo:hi] - q * P, v[lo:hi]
        # order entries [lo_only, mid, hi_only]
        grp = np.where(cq < hi_base, 0, np.where(cq < W, 1, 2))
        o2 = np.argsort(grp, kind="stable")
        cq, rq, vq, grp = cq[o2], rq[o2], vq[o2], grp[o2]
        a = int((grp == 0).sum())
        m = int((grp == 1).sum())
        h = int((grp == 2).sum())
        t = _split_bucket(a, m, h)
        ids = []
        for half, (ce, re, ve) in (
            (0, (cq[: a + t], rq[: a + t], vq[: a + t])),
            (1, (cq[a + t :], rq[a + t :], vq[a + t :])),
        ):
            n = len(ce)
            if n == 0 and not (half == 0 and hi - lo == 0):
                # skip empty half; an entirely empty bucket still emits one
                # all-zero lo sub-chunk so the output tile gets cleared
                continue
            nsc = max(1, _cdiv(n, P))
            pad = nsc * P - n
            base = hi_base if half else 0
            ce = np.concatenate([ce - base, np.zeros(pad, np.int64)])
            re = np.concatenate([re, np.zeros(pad, np.int64)])
            ve = np.concatenate([ve, np.zeros(pad, np.float32)])
            for s in range(nsc):
                ids.append(len(sc_half))
                sc_half.append(half)
                sc_idx.append(ce[s * P : (s + 1) * P])
                sc_rel.append(re[s * P : (s + 1) * P])
                sc_val.append(ve[s * P : (s + 1) * P])
        bucket_sc.append((q, ids))

    pl = Plan()
    pl.NT = NT
    pl.hi_base = hi_base
    pl.S = len(sc_half)
    pl.half = np.asarray(sc_half, np.int64)
    pl.idx = np.stack(sc_idx)  # [S, 128]
    pl.rel = np.stack(sc_rel)
    pl.val = np.stack(sc_val)
    pl.bucket_sc = bucket_sc
    # stream position of each sub-chunk within its half
    pl.pos = np.zeros(pl.S, np.int64)
    for hf in (0, 1):
        sel = pl.half == hf
        pl.pos[sel] = np.arange(sel.sum())
    pl.S_lo = int((pl.half == 0).sum())
    pl.S_hi = int((pl.half == 1).sum())
    return pl


def _build_inputs(pl, x_core):
    """Per-core x2 (batch-interleaved) plus shared constant arrays."""
    bpc, M, F = x_core.shape
    npdt = np.float32
    if DTYPE == "bfloat16":
        import ml_dtypes

        npdt = ml_dtypes.bfloat16
    x2 = np.ascontiguousarray(
        x_core.transpose(1, 0, 2).reshape(M, bpc * F).astype(npdt)
    )

    def idx16(half):
        sel = np.where(pl.half == half)[0]
        if len(sel) == 0:
            return None
        stream = pl.idx[sel].reshape(-1)  # [128 * S_half] in stream order
        n = len(stream)
        arr = np.zeros((16, n // 16), np.int16)
        arr[np.arange(n) % 16, np.arange(n) // 16] = stream.astype(np.int16)
        return np.tile(arr, (8, 1))  # replicate across the 8 q7 cores

    srel = pl.rel.T.astype(np.float32).copy()  # [128, S]
    sval = pl.val.T.astype(np.float32).copy()  # [128, S]
    iota = np.tile(np.arange(P, dtype=np.float32), (P, 1))
    return x2, idx16(0), idx16(1), srel, sval, iota


def _build_nc(M, MP, F, bpc, pl, reps=1):
    S = pl.S
    FB = bpc * F  # interleaved feature width
    dt = mybir.dt
    mmdt = getattr(dt, DTYPE)
    nc = bacc.Bacc(
        "TRN2",
        target_bir_lowering=False,
        debug=False,
        num_devices=NCORES,
        num_swdge_queues=QUEUES,
    )
    x = nc.dram_tensor("x", [M, FB], mmdt, kind="ExternalInput").ap()
    idx_d = [None, None]
    if pl.S_lo:
        idx_d[0] = nc.dram_tensor(
            "idx_lo", [P, 8 * pl.S_lo], dt.int16, kind="ExternalInput"
        ).ap()
    if pl.S_hi:
        idx_d[1] = nc.dram_tensor(
            "idx_hi", [P, 8 * pl.S_hi], dt.int16, kind="ExternalInput"
        ).ap()
    srel = nc.dram_tensor("srel", [P, S], dt.float32, kind="ExternalInput").ap()
    sval = nc.dram_tensor("sval", [P, S], dt.float32, kind="ExternalInput").ap()
    iota = nc.dram_tensor("iota", [P, P], dt.float32, kind="ExternalInput").ap()
    out = nc.dram_tensor("out", [bpc * MP, F], dt.float32, kind="ExternalOutput").ap()

    win = [min(M, W), M - pl.hi_base]  # rows in each gather window
    base = [0, pl.hi_base]
    slen = [pl.S_lo, pl.S_hi]  # stream length (sub-chunks) per half

    with tile.TileContext(nc) as tc:
        with (
            tc.tile_pool(name="const", bufs=1) as cp,
            tc.tile_pool(name="g", bufs=GBUFS) as gp,
            tc.tile_pool(name="sel", bufs=SELBUFS) as selp,
            tc.tile_pool(name="stage", bufs=STBUFS) as stp,
            tc.tile_pool(name="psum", bufs=PSBUFS, space="PSUM") as pp,
        ):
            idx_sb = [None, None]
            for hf in (0, 1):
                if idx_d[hf] is not None:
                    t = cp.tile(list(idx_d[hf].shape), dt.int16, tag=f"idx{hf}")
                    nc.sync.dma_start(out=t[:], in_=idx_d[hf])
                    idx_sb[hf] = t
            srel_sb = cp.tile([P, S], dt.float32)
            nc.sync.dma_start(out=srel_sb[:], in_=srel)
            sval_sb = cp.tile([P, S], dt.float32)
            nc.sync.dma_start(out=sval_sb[:], in_=sval)
            iota_sb = cp.tile([P, P], dt.float32)
            nc.sync.dma_start(out=iota_sb[:], in_=iota)

            op_tiles = [{}, {}]  # per half: op index -> gather tile
            op_counter = [0]

            static_g = [None]

            def ensure_op(hf, o):
                if "staticg" in ABLATE:
                    if static_g[0] is None:
                        sg = gp.tile([P, OPC, FB], mmdt, tag="g",
                                     name="g_static")
                        nc.sync.dma_start(
                            out=sg[:],
                            in_=x[: P * OPC, :].rearrange(
                                "(a p) f -> p a f", p=P
                            ),
                        )
                        static_g[0] = sg
                    op_tiles[hf][o] = static_g[0]
                    return
                if o in op_tiles[hf]:
                    return
                pos0 = o * OPC
                cnt = min(OPC, slen[hf] - pos0)
                gt = gp.tile([P, OPC, FB], mmdt, tag="g",
                             name=f"g{hf}_{o}")
                if "gather" in ABLATE:
                    # dense HWDGE load of the same byte count (timing expt)
                    nc.sync.dma_start(
                        out=gt[:, :cnt, :],
                        in_=x[: P * cnt, :].rearrange(
                            "(a p) f -> p a f", p=P
                        ),
                    )
                else:
                    nc.gpsimd.dma_gather(
                        out_ap=gt[:, :cnt, :],
                        in_ap=x[base[hf] : base[hf] + win[hf], :],
                        idxs_ap=idx_sb[hf][:, 8 * pos0 : 8 * (pos0 + cnt)],
                        num_idxs=P * cnt,
                        num_idxs_reg=P * cnt,
                        elem_size=FB,
                        queue_num=op_counter[0] % QUEUES,
                    )
                    op_counter[0] += 1
                op_tiles[hf][o] = gt

            def body():
                op_tiles[0].clear()
                op_tiles[1].clear()
                if EAGER:
                    order = []
                    for q, ids in pl.bucket_sc:
                        for s in ids:
                            hf = int(pl.half[s])
                            o = int(pl.pos[s]) // OPC
                            if (hf, o) not in order:
                                order.append((hf, o))
                    for hf, o in order:
                        ensure_op(hf, o)
                for q, ids in pl.bucket_sc:
                    ps = pp.tile([P, FB], dt.float32, tag="ps", name=f"ps_{q}")
                    nsc = len(ids)
                    for j, s in enumerate(ids):
                        hf = int(pl.half[s])
                        o, blk = divmod(int(pl.pos[s]), OPC)
                        ensure_op(hf, o)
                        if "compute" in ABLATE:
                            continue
                        sel = selp.tile([P, P], mmdt, tag="sel",
                                        name=f"sel_{s}")
                        nc.vector.tensor_scalar(
                            out=sel[:],
                            in0=iota_sb[:],
                            scalar1=srel_sb[:, s : s + 1],
                            scalar2=sval_sb[:, s : s + 1],
                            op0=mybir.AluOpType.is_equal,
                            op1=mybir.AluOpType.mult,
                        )
                        nc.tensor.matmul(
                            out=ps[:],
                            lhsT=sel[:],
                            rhs=op_tiles[hf][o][:, blk, :],
                            start=(j == 0),
                            stop=(j == nsc - 1),
                        )
                    if "out" in ABLATE:
                        continue
                    stg = stp.tile([P, FB], dt.float32, tag="stage",
                                   name=f"stg_{q}")
                    if "compute" not in ABLATE:
                        nc.vector.tensor_copy(out=stg[:], in_=ps[:])
                    else:
                        nc.vector.memzero(stg[:])
                    rows_q = min(P, MP - q * P)
                    for b in range(bpc):
                        eng = nc.sync if b % 2 == 0 else nc.scalar
                        eng.dma_start(
                            out=out[b * MP + q * P : b * MP + q * P + rows_q, :],
                            in_=stg[:rows_q, b * F : (b + 1) * F],
                        )

            if reps == 1:
                body()
            else:
                with tc.For_i(0, reps, 1):
                    body()
    nc.compile()
    return nc


def _run(x, rows, cols, vals, MP, ncores=NCORES, **run_kwargs):
    B, M, F = x.shape
    assert B % ncores == 0
    bpc = B // ncores
    pl = _build_plan(rows, cols, vals, M, MP)
    nc = _build_nc(M, MP, F, bpc, pl)
    x = np.asarray(x, np.float32)
    in_maps = []
    for r in range(ncores):
        x2, idx_lo, idx_hi, srel, sval, iota = _build_inputs(
            pl, x[r * bpc : (r + 1) * bpc]
        )
        m = {"x": x2, "srel": srel, "sval": sval, "iota": iota}
        if idx_lo is not None:
            m["idx_lo"] = idx_lo
        if idx_hi is not None:
            m["idx_hi"] = idx_hi
        in_maps.append(m)
    res = run_bass_kernel_spmd(
        nc, in_maps, core_ids=list(range(ncores)), **run_kwargs
    )
    out = np.empty((B, MP, F), np.float32)
    for r in range(ncores):
        out[r * bpc : (r + 1) * bpc] = res.results[r]["out"].reshape(bpc, MP, F)
    return out, res, nc


def kernel(x, rows, cols, vals):
    MP = 10242  # number of coarse (pooled) vertices
    out, _, _ = _run(np.asarray(x), np.asarray(rows), np.asarray(cols),
                     np.asarray(vals), MP)
    return out

